# revision 11
# baseline (speedup 1.0000x reference)
"""Causal single-head attention (HeadAttention) for TRN2, 8 NeuronCores.

Reference: q,k,v = x@W (+0 bias); att = softmax(mask(q k^T / 8)); out = att@v.
Shapes: x [4,4096,1024], W [1024,64], out [4,4096,64] fp32.

Sharding (SPMD, one program, per-core data):
  core = (batch b, half h).  Core processes q row-tiles {2s+h : s=0..15}
  (interleaved 128-row tiles) -> causal work is balanced: slot s always
  attends key-tiles [0, 2s+2), with a per-core 128x256 additive mask
  making the last two key-tiles causal (h=0: [diag, -inf]; h=1: [0, diag]).

Per-core pipeline:
  PE-transpose x row-tiles -> x^T; project k^T[64,4096], v[4096,64+ones],
  q^T[64,2048] (scaled by 1/8); scores[128,512] blocks = q^T.T @ k^T in
  PSUM; mask-add; exp PSUM->SBUF; PE-transpose P tiles; O = sum P^T.T @
  v_aug accumulated in PSUM; normalize by the appended ones-column sum.
"""

import sys

sys.path.insert(0, "/opt/trn_rl_repo")

import numpy as np

import concourse.bass as bass
import concourse.mybir as mybir
import concourse.tile as tile
from concourse import bacc
from concourse.bass_utils import run_bass_kernel_spmd
from concourse.masks import make_identity

B, T, C, H = 4, 4096, 1024, 64
P = 128
NT_Q = 16          # q row-tiles per core
NT_K = T // P      # 32 key tiles
CO = C // P        # 8 contraction chunks
TQ = NT_Q * P      # 2048 q rows per core
NEG = -1.0e9
FP32 = mybir.dt.float32


def _build_program():
    nc = bacc.Bacc()
    xq = nc.dram_tensor("xq", [TQ, C], FP32, kind="ExternalInput").ap()
    xkv = nc.dram_tensor("xkv", [T, C], FP32, kind="ExternalInput").ap()
    wq = nc.dram_tensor("wq", [C, H], FP32, kind="ExternalInput").ap()
    wk = nc.dram_tensor("wk", [C, H], FP32, kind="ExternalInput").ap()
    wv = nc.dram_tensor("wv", [C, H], FP32, kind="ExternalInput").ap()
    maskadd = nc.dram_tensor("maskadd", [P, 2 * P], FP32,
                             kind="ExternalInput").ap()
    out = nc.dram_tensor("out", [TQ, H], FP32, kind="ExternalOutput").ap()

    with tile.TileContext(nc) as tc:
        with (
            tc.tile_pool(name="const", bufs=1) as const,
            tc.tile_pool(name="persist", bufs=1) as persist,
            tc.tile_pool(name="xload", bufs=3) as xload,
            tc.tile_pool(name="xtp", bufs=3) as xtp,
            tc.tile_pool(name="pbuf", bufs=2) as pbuf,
            tc.tile_pool(name="ptb", bufs=4) as ptb,
            tc.tile_pool(name="small", bufs=4) as small,
            tc.tile_pool(name="psT", bufs=2, space="PSUM") as psT,
            tc.tile_pool(name="psS", bufs=2, space="PSUM") as psS,
            tc.tile_pool(name="psP", bufs=1, space="PSUM") as psP,
            tc.tile_pool(name="psO", bufs=2, space="PSUM") as psO,
        ):
            ident = const.tile([P, P], FP32)
            make_identity(nc, ident)
            mask_sb = const.tile([P, 2 * P], FP32)
            nc.sync.dma_start(mask_sb, maskadd)

            w_sb = {}
            for name, w in (("q", wq), ("k", wk), ("v", wv)):
                t = const.tile([P, CO, H], FP32, tag=f"w{name}")
                nc.sync.dma_start(t, w.rearrange("(o p) h -> p o h", p=P))
                w_sb[name] = t

            kT_sb = persist.tile([H, T], FP32, tag="kT")
            v_sb = persist.tile([P, NT_K, H + 1], FP32, tag="v")
            qT_sb = persist.tile([H, TQ], FP32, tag="qT")
            # ones column of v_aug gives the softmax denominator for free
            nc.any.memset(v_sb[:, :, H : H + 1], 1.0)

            def xT_tile(src, rt):
                """Load 128 rows of src, return [128c, CO, 128rows] SBUF x^T."""
                xt = xload.tile([P, C], FP32, tag="xt")
                nc.sync.dma_start(xt, src[rt * P : (rt + 1) * P, :])
                xT = xtp.tile([P, CO, P], FP32, tag="xT")
                for o in range(CO):
                    ps = psT.tile([P, P], FP32, tag="t")
                    nc.tensor.transpose(ps, xt[:, o * P : (o + 1) * P], ident)
                    nc.vector.tensor_copy(xT[:, o, :], ps)
                return xT

            # k^T, v (+ ones col) over all 32 key tiles
            for kt in range(NT_K):
                xT = xT_tile(xkv, kt)
                pk = psP.tile([H, P], FP32, tag="pk")
                pv = psP.tile([P, H], FP32, tag="pv")
                for o in range(CO):
                    nc.tensor.matmul(pk, w_sb["k"][:, o, :], xT[:, o, :],
                                     start=(o == 0), stop=(o == CO - 1))
                for o in range(CO):
                    nc.tensor.matmul(pv, xT[:, o, :], w_sb["v"][:, o, :],
                                     start=(o == 0), stop=(o == CO - 1))
                nc.vector.tensor_copy(kT_sb[:, kt * P : (kt + 1) * P], pk)
                nc.vector.tensor_copy(v_sb[:, kt, :H], pv)

            # q^T for this core's 16 row tiles (1/sqrt(H) folded into Wq host-side)
            for rt in range(NT_Q):
                xT = xT_tile(xq, rt)
                pq = psP.tile([H, P], FP32, tag="pk")
                for o in range(CO):
                    nc.tensor.matmul(pq, w_sb["q"][:, o, :], xT[:, o, :],
                                     start=(o == 0), stop=(o == CO - 1))
                nc.vector.tensor_copy(qT_sb[:, rt * P : (rt + 1) * P], pq)

            # attention per slot
            for s in range(NT_Q):
                KS = (2 * s + 2) * P          # keys attended this slot
                nch = (KS + 511) // 512
                p_sb = pbuf.tile([P, T], FP32, tag="p")
                for ch in range(nch):
                    w = min(512, KS - ch * 512)
                    ps = psS.tile([P, 512], FP32, tag="s")
                    nc.tensor.matmul(ps[:, :w], qT_sb[:, s * P : (s + 1) * P],
                                     kT_sb[:, ch * 512 : ch * 512 + w],
                                     start=True, stop=True)
                    if ch == nch - 1:
                        nc.vector.tensor_tensor(
                            ps[:, w - 256 : w], ps[:, w - 256 : w], mask_sb,
                            mybir.AluOpType.add)
                    nc.scalar.activation(p_sb[:, ch * 512 : ch * 512 + w],
                                         ps[:, :w],
                                         mybir.ActivationFunctionType.Exp)
                po = psO.tile([P, H + 1], FP32, tag="o")
                nk = KS // P
                for kt in range(nk):
                    pt_ps = psT.tile([P, P], FP32, tag="t")
                    nc.tensor.transpose(pt_ps, p_sb[:, kt * P : (kt + 1) * P],
                                        ident)
                    pt_sb = ptb.tile([P, P], FP32, tag="pt")
                    nc.vector.tensor_copy(pt_sb, pt_ps)
                    nc.tensor.matmul(po, pt_sb, v_sb[:, kt, :],
                                     start=(kt == 0), stop=(kt == nk - 1))
                rin = small.tile([P, 1], FP32, tag="rin")
                nc.vector.reciprocal(rin, po[:, H : H + 1])
                o_sb = small.tile([P, H], FP32, tag="osb")
                nc.vector.tensor_tensor(o_sb, po[:, :H],
                                        rin.to_broadcast((P, H)),
                                        mybir.AluOpType.mult)
                nc.sync.dma_start(out[s * P : (s + 1) * P, :], o_sb)
    nc.finalize()
    return nc


_NC = None


def kernel(x, mask, Wq, bq, Wk, bk, Wv, bv):
    global _NC
    x = np.ascontiguousarray(np.asarray(x, dtype=np.float32))
    # attention scale folded into Wq (1/8 is exact in fp32)
    Wq = np.asarray(Wq, dtype=np.float32) * np.float32(1.0 / np.sqrt(H))
    Wk = np.asarray(Wk, dtype=np.float32)
    Wv = np.asarray(Wv, dtype=np.float32)

    # per-half additive masks for the last two key-tiles of every slot
    diag = np.where(np.triu(np.ones((P, P), dtype=bool), k=1), NEG, 0.0)
    diag = diag.astype(np.float32)
    m0 = np.concatenate([diag, np.full((P, P), NEG, np.float32)], axis=1)
    m1 = np.concatenate([np.zeros((P, P), np.float32), diag], axis=1)
    masks = [m0, m1]

    xt = x.reshape(B, NT_K, P, C)
    in_maps = []
    for b in range(B):
        for h in range(2):
            idx = [2 * s + h for s in range(NT_Q)]
            in_maps.append({
                "xq": np.ascontiguousarray(
                    xt[b, idx].reshape(TQ, C)),
                "xkv": x[b],
                "wq": Wq, "wk": Wk, "wv": Wv,
                "maskadd": masks[h],
            })

    if _NC is None:
        _NC = _build_program()
    res = run_bass_kernel_spmd(_NC, in_maps, core_ids=list(range(8)))

    out = np.empty((B, NT_K, P, H), dtype=np.float32)
    for b in range(B):
        for h in range(2):
            idx = [2 * s + h for s in range(NT_Q)]
            out[b, idx] = res.results[b * 2 + h]["out"].reshape(NT_Q, P, H)
    return out.reshape(B, T, H)
